# revision 9
# baseline (speedup 1.0000x reference)
"""Trainium2 Bass kernel for a 2-layer GCN over 2048 independent 25-node
KNN subgraphs (gnn_message_passing).

Strategy (v5, aggregate-first, LDWEIGHTS-port-lean, stall-free pipeline):
  - Each 25-node subgraph is independent -> the sparse aggregation is a
    dense per-graph 25x25 matmul. Host packs the normalized adjacency
    into block-diagonal 128x128 tiles (5 graphs per tile), bf16.
  - Layer 1 aggregate-first:  zT = x.T @ at;  h1 = relu(zT.T @ W0).
  - Layer-2 center aggregation: p2 = h1.T @ atc (2 matmuls/tile, 5 mov
    cols) into block-persistent PSUM banks; W1+Wlin once per block
    (2 blocks), reusing the block's own bank for h3/o after its copy.
  - Pipeline (per iteration b): mmA(b+1) -> cast zt(b+1) [vector] ->
    h1(b) -> relu(b) [scalar 224 cols + vector 32] -> p2(b-1).
    p2 is deferred one batch so it never waits on relu; the cast runs
    one batch ahead so h1 never waits on vector.  Steady state is
    LDWEIGHTS-port-bound (~16 x 95ns per 4-tile batch).
  - PSUM: zt 2 banks (bufs=2), h1 2x2 banks (bufs=2), p2 2 banks. = 8.
  - DMA: few big issues (a dma_start costs ~600-750ns on its engine);
    x + late-at on the sync HW ring, w0/atc/mid-at on the scalar HW
    ring (both stream in parallel), packed w1|wl on gpsimd.
  - 4 warm-up matmuls bridge PE activity from the init barrier until
    real data arrives, so the HAM k=4->k=8 clock gate opens during the
    DMA head.
  - Data parallel over 8 cores: 256 graphs (52 tiles) per core.
"""

import os
import sys

import ml_dtypes
import numpy as np

for _p in ("/opt/trn_rl_repo", "/opt/trn_rl_repo/concourse"):
    if _p not in sys.path:
        sys.path.insert(0, _p)

import concourse.bass as bass
import concourse.tile as tile
from concourse import bacc, mybir
from concourse.bass_utils import run_bass_kernel_spmd

NCORES = 8
B = 2048            # graphs
K = 25              # nodes per graph
N = B * K           # 51200
GPC = B // NCORES   # 256 graphs per core
G = 5               # graphs packed per PE tile
P = G * K           # 125 real partitions per tile
PP = 128            # padded partition count
NT = (GPC + G - 1) // G   # 52 tiles per core (last tile: 1 real graph)
CP = 5              # centers per tile
AW = 128            # adjacency tile width (125 block cols + 3 zero pad)
F0 = 128            # input features
F1 = 256            # hidden features
TB = 4              # tiles per batch
NB = NT // TB       # 13 batches
BB = [0, 32, 52]    # W1/Wlin block bounds (block 0 flushes mid-loop)
B1S = 48            # block-1 split: tiles [32:48] flush before p2(12)
VSPLIT = 32         # relu cols done on vector engine (rest on scalar)
NWARM = 16          # PE warm-up matmuls: a dense ~7us (k=4) stream that
                    # opens the HAM clock gate during the DMA head, so
                    # real matmuls run at k=8 (measured: gate opens after
                    # ~4.6us of sustained activity; a sparse ramp never
                    # opens it and the whole loop runs at half clock)

_f32 = mybir.dt.float32
_bf16 = mybir.dt.bfloat16

_compiled = {}


def _build_nc():
    nc = bacc.Bacc("TRN2", target_bir_lowering=False, debug=False,
                   num_devices=NCORES)

    x_d = nc.dram_tensor("x", [PP, NT, F0], _bf16, kind="ExternalInput")
    at_d = nc.dram_tensor("at", [PP, NT, AW], _bf16, kind="ExternalInput")
    atc_d = nc.dram_tensor("atc", [PP, NT, CP], _bf16, kind="ExternalInput")
    w0_d = nc.dram_tensor("w0", [F0, F1], _bf16, kind="ExternalInput")
    wpk_d = nc.dram_tensor("wpk", [128, 2 * F1 + 2], _bf16,
                           kind="ExternalInput")
    out_d = nc.dram_tensor("out", [1, NT * CP], _f32, kind="ExternalOutput")

    relu = mybir.ActivationFunctionType.Relu

    with tile.TileContext(nc) as tc:
        with (
            tc.tile_pool(name="const", bufs=1) as cpool,
            tc.tile_pool(name="ztp", bufs=3) as ztp,
            tc.tile_pool(name="h1p", bufs=3) as h1p,
            tc.tile_pool(name="p2p", bufs=2) as p2p,
            tc.tile_pool(name="h3p", bufs=2) as h3p,
            tc.tile_pool(name="outp", bufs=1) as outp,
            tc.tile_pool(name="psum", bufs=1, space=bass.MemorySpace.PSUM) as psp,
        ):
            w0 = cpool.tile([F0, F1], _bf16)
            x_sb = cpool.tile([PP, NT, F0], _bf16)
            at_sb = cpool.tile([PP, NT, AW], _bf16)
            atc_sb = cpool.tile([PP, NT, CP], _bf16)
            wpk = cpool.tile([128, 2 * F1 + 2], _bf16)
            scratch = cpool.tile([128, 512], _bf16)
            out_sb = outp.tile([1, NT * CP], _f32)

            nc.gpsimd.memset(scratch[:], 0.0)

            # ---- DMA issues (~600-750ns each on the issuing engine).
            # sync HW ring: x + tail at; scalar HW ring: w0, atc, mid at
            # (streams in parallel with sync's ring, and is done before
            # scalar's first relu); gpsimd software ring: packed w1|wl.
            nc.sync.dma_start(x_sb[:, 0:4, :], x_d[:, 0:4, :])
            nc.scalar.dma_start(w0[:], w0_d[:])
            nc.sync.dma_start(at_sb[:, 0:4, :], at_d[:, 0:4, :])
            nc.scalar.dma_start(atc_sb[:], atc_d[:])
            nc.sync.dma_start(x_sb[:, 4:12, :], x_d[:, 4:12, :])
            nc.scalar.dma_start(at_sb[:, 4:12, :], at_d[:, 4:12, :])
            nc.sync.dma_start(x_sb[:, 12:22, :], x_d[:, 12:22, :])
            nc.scalar.dma_start(at_sb[:, 12:22, :], at_d[:, 12:22, :])
            nc.sync.dma_start(x_sb[:, 22:36, :], x_d[:, 22:36, :])
            nc.scalar.dma_start(at_sb[:, 22:36, :], at_d[:, 22:36, :])
            nc.sync.dma_start(x_sb[:, 36:52, :], x_d[:, 36:52, :])
            nc.sync.dma_start(at_sb[:, 36:52, :], at_d[:, 36:52, :])
            nc.gpsimd.dma_start(wpk[:], wpk_d[:])

            # ---- PSUM layout (8 banks):
            #   zt  tag: 2 x [128,4,128] f32 (1 bank each)
            #   h1  tag: 2 x [128,4,256] f32 (2 banks each; warm shares)
            #   p2  tag: 2 x [128,512] f32 (block p2 + h3/o reuse)
            p2bank = [psp.tile([128, 512], _f32, tag="p2", bufs=2,
                               name=f"p2bank{k}") for k in range(2)]

            warm_ps = psp.tile([128, TB, F1], _f32, tag="h1", bufs=2)
            for _ in range(NWARM):
                nc.tensor.matmul(warm_ps[:, 0:2, :], scratch[:, 0:128],
                                 scratch[:], start=True, stop=True)

            pending = {}

            def defer(b, fn):
                pending.setdefault(b, []).append(fn)

            def emit_mma(b):
                zt_ps = psp.tile([128, TB, 128], _f32, tag="zt", bufs=2)
                for j in range(TB):
                    i = b * TB + j
                    nc.tensor.matmul(zt_ps[:, j, :], x_sb[:, i, :],
                                     at_sb[:, i, :], start=True, stop=True)
                return zt_ps

            def emit_p2(b):
                # layer-2 center aggregation for batch b (deferred one
                # batch so it never waits on relu)
                for jj in range(TB):
                    i = b * TB + jj
                    k = 0 if i < BB[1] else 1
                    bsz5 = (BB[k + 1] - BB[k]) * CP
                    off = (i - BB[k]) * CP
                    for fic in range(2):
                        nc.tensor.matmul(
                            p2bank[k][:, fic * bsz5 + off:
                                      fic * bsz5 + off + CP],
                            h1_sbs[b % 3][:, jj, fic * 128:(fic + 1) * 128],
                            atc_sb[:, i, :],
                            start=True, stop=True)
                    if i + 1 == BB[1]:
                        emit_flush0()

            def w1_mm(bank, p2_sb, bsz5, clo, chi):
                # h3[:, foc, clo:chi] += W1 chunks @ p2 (h3 reuses the
                # block's own bank; its p2 cols are dead after the copy)
                for foc in range(2):
                    for fic in range(2):
                        nc.tensor.matmul(
                            bank[:, foc * bsz5 + clo:foc * bsz5 + chi],
                            wpk[:, fic * F1 + foc * 128:
                                fic * F1 + (foc + 1) * 128],
                            p2_sb[:, fic * bsz5 + clo:fic * bsz5 + chi],
                            start=(fic == 0), stop=(fic == 1))

            def relu_wl_out(bank, bsz5, lo, hi):
                h3_sb = h3p.tile([128, 2 * bsz5], _bf16, name="h3_sb")
                nc.scalar.activation(h3_sb[:], bank[:, 0:2 * bsz5], relu)
                o_ps = bank[0:1, 2 * bsz5:3 * bsz5]
                for foc in range(2):
                    nc.tensor.matmul(o_ps, wpk[:, 2 * F1 + foc:
                                               2 * F1 + foc + 1],
                                     h3_sb[:, foc * bsz5:(foc + 1) * bsz5],
                                     start=(foc == 0), stop=(foc == 1))
                nc.vector.tensor_copy(out_sb[0:1, lo * CP:hi * CP], o_ps)

            def emit_flush0():
                # block 0 complete: pipeline split-copy -> W1 -> out over
                # the next iterations, all inside the loop
                bsz5 = BB[1] * CP
                bank = p2bank[0]
                cell = {}

                def cp(c0, c1):
                    def fn():
                        if "p2_sb" not in cell:
                            cell["p2_sb"] = p2p.tile([128, 2 * bsz5], _bf16,
                                                     name="p2a_sb")
                        nc.vector.tensor_copy(cell["p2_sb"][:, c0:c1],
                                              bank[:, c0:c1])
                    return fn

                base = cur_b[0]
                defer(base, cp(0, bsz5))
                defer(base + 1, cp(bsz5, 2 * bsz5))
                defer(base + 2, lambda: w1_mm(bank, cell["p2_sb"], bsz5,
                                              0, bsz5))
                defer(base + 3, lambda: relu_wl_out(bank, bsz5, 0, BB[1]))

            # ---- software-pipelined main loop ----
            h1_sbs = {}
            cur_b = [0]
            zt_sbs = {}
            zt_ps = emit_mma(0)
            zt_sbs[0] = ztp.tile([128, TB, 128], _bf16, name="zt_sb")
            nc.vector.tensor_copy(zt_sbs[0][:], zt_ps[:])

            for b in range(NB):
                cur_b[0] = b
                if b + 1 < NB:
                    zt_ps = emit_mma(b + 1)
                    zt_sbs[(b + 1) % 3] = ztp.tile([128, TB, 128], _bf16,
                                                   name="zt_sb")
                    nc.vector.tensor_copy(zt_sbs[(b + 1) % 3][:], zt_ps[:])

                zt_sb = zt_sbs[b % 3]
                h1_ps = psp.tile([128, TB, F1], _f32, tag="h1", bufs=2)
                for j in range(TB):
                    nc.tensor.matmul(h1_ps[:, j, :], zt_sb[:, j, :], w0[:],
                                     start=True, stop=True)
                h1_sb = h1p.tile([128, TB, F1], _bf16)
                h1_sbs[b % 3] = h1_sb
                nc.scalar.activation(h1_sb[:, :, VSPLIT:F1],
                                     h1_ps[:, :, VSPLIT:F1], relu)
                nc.vector.tensor_scalar_max(h1_sb[:, :, 0:VSPLIT],
                                            h1_ps[:, :, 0:VSPLIT], 0.0)
                if b > 0:
                    emit_p2(b - 1)
                if b == NB - 1:
                    # first piece of block 1's p2 copy (tiles [32:48] are
                    # done after emit_p2(11); cols [80:100] are written
                    # later by p2(12) and re-copied in the tail)
                    bsz5b = (NT - BB[1]) * CP
                    c1 = (B1S - BB[1]) * CP
                    p2b_sb = p2p.tile([128, 2 * bsz5b], _bf16,
                                      name="p2b_sb")
                    nc.vector.tensor_copy(p2b_sb[:, 0:bsz5b + c1],
                                          p2bank[1][:, 0:bsz5b + c1])
                for fn in pending.pop(b, []):
                    fn()

            # ---- tail: W1 piece 1 (tiles 32:48) overlaps the last p2 ----
            cur_b[0] = NB
            bsz5b = (NT - BB[1]) * CP
            c1 = (B1S - BB[1]) * CP
            w1_mm(p2bank[1], p2b_sb, bsz5b, 0, c1)
            emit_p2(NB - 1)                      # tiles 48-51
            nc.vector.tensor_copy(p2b_sb[:, c1:bsz5b],
                                  p2bank[1][:, c1:bsz5b])
            nc.vector.tensor_copy(p2b_sb[:, bsz5b + c1:2 * bsz5b],
                                  p2bank[1][:, bsz5b + c1:2 * bsz5b])
            w1_mm(p2bank[1], p2b_sb, bsz5b, c1, bsz5b)
            relu_wl_out(p2bank[1], bsz5b, BB[1], NT)
            nc.sync.dma_start(out_d[:], out_sb[:])
            for b in sorted(list(pending)):
                for fn in pending.pop(b):
                    fn()

    nc.compile()
    return nc


def _get_nc(mode=None):
    if "v5" not in _compiled:
        _compiled["v5"] = _build_nc()
    return _compiled["v5"]


def _host_prep(x, edge_weight, W0, W1, Wlin, edge_index):
    bf = ml_dtypes.bfloat16
    src = edge_index[0].astype(np.int64)
    tgt = edge_index[1].astype(np.int64)
    b = src // K
    sl = src - b * K
    tl = tgt - (tgt // K) * K

    # dense raw adjacency per graph, indexed [b, t, s]
    idx = (b * K + tl) * K + sl
    Araw = np.bincount(idx, weights=edge_weight.astype(np.float64),
                       minlength=B * K * K).astype(np.float32).reshape(B, K, K)
    deg = Araw.sum(axis=2)                      # weighted in-degree [B, K]
    with np.errstate(divide="ignore"):
        dinv = np.where(deg > 0, 1.0 / np.sqrt(deg), 0.0).astype(np.float32)
    An = Araw * dinv[:, :, None] * dinv[:, None, :]   # [b, t, s]
    ATn = np.ascontiguousarray(An.transpose(0, 2, 1))  # [b, s, t]

    # scatter graphs into per-core padded tile slots
    SLOTS = NT * G
    ATs = np.zeros((NCORES, SLOTS, K, K), np.float32)
    ATs[:, :GPC] = ATn.reshape(NCORES, GPC, K, K)
    ATs = ATs.reshape(NCORES, NT, G, K, K)

    at = np.zeros((NCORES, NT, PP, AW), np.float32)
    bd = at[:, :, :P, :P].reshape(NCORES, NT, G, K, G, K)
    atc = np.zeros((NCORES, NT, PP, CP), np.float32)
    cent = atc[:, :, :P, :G].reshape(NCORES, NT, G, K, G)
    for g in range(G):
        bd[:, :, g, :, g, :] = ATs[:, :, g]          # block-diagonal AT
        cent[:, :, g, :, g] = ATs[:, :, g, :, 0]     # center (t_local=0) col
    # device layout [PP, NT, .]
    at = np.ascontiguousarray(at.transpose(0, 2, 1, 3).astype(bf))
    atc = np.ascontiguousarray(atc.transpose(0, 2, 1, 3).astype(bf))

    # node-major x, tiled and padded: x_nm[p, i, f] = x[i*P + p, f], p < 125
    xp = np.zeros((NCORES, NT, PP, F0), np.float32)
    xtmp = np.zeros((NCORES, NT * P, F0), np.float32)
    xtmp[:, :GPC * K] = x.reshape(NCORES, GPC * K, F0)
    xp[:, :, :P, :] = xtmp.reshape(NCORES, NT, P, F0)
    x_nm = np.ascontiguousarray(xp.transpose(0, 2, 1, 3).astype(bf))

    # packed [w1 fic0 | w1 fic1 | wl]: wpk[p, fic*256+fo] = W1[fic*128+p, fo]
    wpk = np.empty((128, 2 * F1 + 2), np.float32)
    w1p = W1.reshape(2, 128, F1).transpose(1, 0, 2)   # [128, fic, fo]
    wpk[:, 0:F1] = w1p[:, 0, :]
    wpk[:, F1:2 * F1] = w1p[:, 1, :]
    wpk[:, 2 * F1:] = Wlin.reshape(2, 128).T          # [128, foc]
    wpk = np.ascontiguousarray(wpk.astype(bf))

    in_maps = []
    for c in range(NCORES):
        in_maps.append({
            "x": x_nm[c],
            "at": np.ascontiguousarray(at[c]),
            "atc": np.ascontiguousarray(atc[c]),
            "w0": np.ascontiguousarray(W0.astype(bf)),
            "wpk": wpk,
        })
    return in_maps


def _run(inputs, mode=None, trace=False):
    nc = _get_nc()
    in_maps = _host_prep(**inputs)
    res = run_bass_kernel_spmd(nc, in_maps, core_ids=list(range(NCORES)),
                               trace=trace)
    out = np.empty((B, 1), np.float32)
    for c in range(NCORES):
        vals = res.results[c]["out"].reshape(-1)
        out[c * GPC:(c + 1) * GPC, 0] = vals[:GPC]
    return out, res


def kernel(**inputs):
    out, _ = _run(inputs, trace=False)
    return out


# revision 13
# speedup vs baseline: 1.0184x; 1.0184x over previous
"""Trainium2 Bass kernel for a 2-layer GCN over 2048 independent 25-node
KNN subgraphs (gnn_message_passing).

Strategy (v5, aggregate-first, LDWEIGHTS-port-lean, stall-free pipeline):
  - Each 25-node subgraph is independent -> the sparse aggregation is a
    dense per-graph 25x25 matmul. Host packs the normalized adjacency
    into block-diagonal 128x128 tiles (5 graphs per tile), bf16.
  - Layer 1 aggregate-first:  zT = x.T @ at;  h1 = relu(zT.T @ W0).
  - Layer-2 center aggregation: p2 = h1.T @ atc (2 matmuls/tile, 5 mov
    cols) into block-persistent PSUM banks; W1+Wlin once per block
    (2 blocks), reusing the block's own bank for h3/o after its copy.
  - Pipeline (per iteration b): mmA(b+1) -> cast zt(b+1) [vector] ->
    h1(b) -> relu(b) [scalar 224 cols + vector 32] -> p2(b-1).
    p2 is deferred one batch so it never waits on relu; the cast runs
    one batch ahead so h1 never waits on vector.  Steady state is
    LDWEIGHTS-port-bound (~16 x 95ns per 4-tile batch).
  - PSUM: zt 2 banks (bufs=2), h1 2x2 banks (bufs=2), p2 2 banks. = 8.
  - DMA: few big issues (a dma_start costs ~600-750ns on its engine);
    x + late-at on the sync HW ring, w0/atc/mid-at on the scalar HW
    ring (both stream in parallel), packed w1|wl on gpsimd.
  - 4 warm-up matmuls bridge PE activity from the init barrier until
    real data arrives, so the HAM k=4->k=8 clock gate opens during the
    DMA head.
  - Data parallel over 8 cores: 256 graphs (52 tiles) per core.
"""

import os
import sys

import ml_dtypes
import numpy as np

for _p in ("/opt/trn_rl_repo", "/opt/trn_rl_repo/concourse"):
    if _p not in sys.path:
        sys.path.insert(0, _p)

import concourse.bass as bass
import concourse.tile as tile
from concourse import bacc, mybir
from concourse.bass_utils import run_bass_kernel_spmd

NCORES = 8
B = 2048            # graphs
K = 25              # nodes per graph
N = B * K           # 51200
GPC = B // NCORES   # 256 graphs per core
G = 5               # graphs packed per PE tile
P = G * K           # 125 real partitions per tile
PP = 128            # padded partition count
NT = (GPC + G - 1) // G   # 52 tiles per core (last tile: 1 real graph)
CP = 5              # centers per tile
AW = 128            # adjacency tile width (125 block cols + 3 zero pad)
F0 = 128            # input features
F1 = 256            # hidden features
TB = 4              # tiles per batch
NB = NT // TB       # 13 batches
BB = [0, 32, 52]    # W1/Wlin block bounds (block 0 flushes mid-loop)
B1S = 48            # block-1 split: tiles [32:48] flush before p2(12)
VSPLIT = 32         # relu cols done on vector engine (rest on scalar)
NWARM = 6           # pre-loop PE warm-up matmuls (n=512); plus 2 filler
                    # matmuls in each of the first 5 iterations.  The HAM
                    # clock gate (k=4 -> k=8) opens only after ~4.6us of
                    # dense PE activity; a sparse ramp never opens it and
                    # the whole loop runs at half clock.

_f32 = mybir.dt.float32
_bf16 = mybir.dt.bfloat16

_compiled = {}


def _build_nc():
    nc = bacc.Bacc("TRN2", target_bir_lowering=False, debug=False,
                   num_devices=NCORES)

    x_d = nc.dram_tensor("x", [PP, NT, F0], _bf16, kind="ExternalInput")
    at_d = nc.dram_tensor("at", [PP, NT, AW], _bf16, kind="ExternalInput")
    atc_d = nc.dram_tensor("atc", [PP, NT, CP], _bf16, kind="ExternalInput")
    w0_d = nc.dram_tensor("w0", [F0, F1], _bf16, kind="ExternalInput")
    wpk_d = nc.dram_tensor("wpk", [128, 2 * F1 + 2], _bf16,
                           kind="ExternalInput")
    out_d = nc.dram_tensor("out", [1, NT * CP], _f32, kind="ExternalOutput")

    relu = mybir.ActivationFunctionType.Relu

    with tile.TileContext(nc) as tc:
        with (
            tc.tile_pool(name="const", bufs=1) as cpool,
            tc.tile_pool(name="ztp", bufs=3) as ztp,
            tc.tile_pool(name="h1p", bufs=3) as h1p,
            tc.tile_pool(name="p2p", bufs=2) as p2p,
            tc.tile_pool(name="h3p", bufs=2) as h3p,
            tc.tile_pool(name="outp", bufs=1) as outp,
            tc.tile_pool(name="psum", bufs=1, space=bass.MemorySpace.PSUM) as psp,
        ):
            w0 = cpool.tile([F0, F1], _bf16)
            x_sb = cpool.tile([PP, NT, F0], _bf16)
            at_sb = cpool.tile([PP, NT, AW], _bf16)
            atc_sb = cpool.tile([PP, NT, CP], _bf16)
            wpk = cpool.tile([128, 2 * F1 + 2], _bf16)
            scratch = cpool.tile([128, 512], _bf16)
            out_sb = outp.tile([1, NT * CP], _f32)

            nc.gpsimd.memset(scratch[:], 0.0)

            # ---- DMA issues (~600-750ns each on the issuing engine).
            # sync HW ring: x + tail at; scalar HW ring: w0, atc, mid at
            # (streams in parallel with sync's ring, and is done before
            # scalar's first relu); gpsimd software ring: packed w1|wl.
            nc.sync.dma_start(x_sb[:, 0:4, :], x_d[:, 0:4, :])
            nc.scalar.dma_start(w0[:], w0_d[:])
            nc.sync.dma_start(at_sb[:, 0:4, :], at_d[:, 0:4, :])
            nc.scalar.dma_start(atc_sb[:], atc_d[:])
            nc.sync.dma_start(x_sb[:, 4:18, :], x_d[:, 4:18, :])
            nc.scalar.dma_start(at_sb[:, 4:18, :], at_d[:, 4:18, :])
            nc.sync.dma_start(x_sb[:, 18:34, :], x_d[:, 18:34, :])
            nc.scalar.dma_start(at_sb[:, 18:34, :], at_d[:, 18:34, :])
            nc.sync.dma_start(x_sb[:, 34:52, :], x_d[:, 34:52, :])
            nc.sync.dma_start(at_sb[:, 34:52, :], at_d[:, 34:52, :])
            nc.gpsimd.dma_start(wpk[:], wpk_d[:])

            # ---- PSUM layout (8 banks):
            #   zt  tag: 2 x [128,4,128] f32 (1 bank each)
            #   h1  tag: 2 x [128,4,256] f32 (2 banks each; warm shares)
            #   p2  tag: 2 x [128,512] f32 (block p2 + h3/o reuse)
            p2bank = [psp.tile([128, 512], _f32, tag="p2", bufs=2,
                               name=f"p2bank{k}") for k in range(2)]

            warm_ps = psp.tile([128, TB, F1], _f32, tag="h1", bufs=2)

            def warm(n):
                # dense PE activity to open (and hold open) the HAM
                # clock gate; also fills would-be DMA stalls in the ramp
                for _ in range(n):
                    nc.tensor.matmul(warm_ps[:, 0:2, :], scratch[:, 0:128],
                                     scratch[:], start=True, stop=True)

            warm(6)

            pending = {}

            def defer(b, fn):
                pending.setdefault(b, []).append(fn)

            def emit_mma(b):
                zt_ps = psp.tile([128, TB, 128], _f32, tag="zt", bufs=2)
                for j in range(TB):
                    i = b * TB + j
                    nc.tensor.matmul(zt_ps[:, j, :], x_sb[:, i, :],
                                     at_sb[:, i, :], start=True, stop=True)
                return zt_ps

            def emit_p2(b):
                # layer-2 center aggregation for batch b (deferred one
                # batch so it never waits on relu)
                for jj in range(TB):
                    i = b * TB + jj
                    k = 0 if i < BB[1] else 1
                    bsz5 = (BB[k + 1] - BB[k]) * CP
                    off = (i - BB[k]) * CP
                    for fic in range(2):
                        nc.tensor.matmul(
                            p2bank[k][:, fic * bsz5 + off:
                                      fic * bsz5 + off + CP],
                            h1_sbs[b % 3][:, jj, fic * 128:(fic + 1) * 128],
                            atc_sb[:, i, :],
                            start=True, stop=True)
                    if i + 1 == BB[1]:
                        emit_flush0()

            def w1_mm(bank, p2_sb, bsz5, clo, chi):
                # h3[:, foc, clo:chi] += W1 chunks @ p2 (h3 reuses the
                # block's own bank; its p2 cols are dead after the copy)
                for foc in range(2):
                    for fic in range(2):
                        nc.tensor.matmul(
                            bank[:, foc * bsz5 + clo:foc * bsz5 + chi],
                            wpk[:, fic * F1 + foc * 128:
                                fic * F1 + (foc + 1) * 128],
                            p2_sb[:, fic * bsz5 + clo:fic * bsz5 + chi],
                            start=(fic == 0), stop=(fic == 1))

            def relu_wl_out(bank, bsz5, lo, hi):
                h3_sb = h3p.tile([128, 2 * bsz5], _bf16, name="h3_sb")
                nc.scalar.activation(h3_sb[:], bank[:, 0:2 * bsz5], relu)
                o_ps = bank[0:1, 2 * bsz5:3 * bsz5]
                for foc in range(2):
                    nc.tensor.matmul(o_ps, wpk[:, 2 * F1 + foc:
                                               2 * F1 + foc + 1],
                                     h3_sb[:, foc * bsz5:(foc + 1) * bsz5],
                                     start=(foc == 0), stop=(foc == 1))
                nc.vector.tensor_copy(out_sb[0:1, lo * CP:hi * CP], o_ps)

            def emit_flush0():
                # block 0 complete: pipeline split-copy -> W1 -> out over
                # the next iterations, all inside the loop
                bsz5 = BB[1] * CP
                bank = p2bank[0]
                cell = {}

                def cp(c0, c1):
                    def fn():
                        if "p2_sb" not in cell:
                            cell["p2_sb"] = p2p.tile([128, 2 * bsz5], _bf16,
                                                     name="p2a_sb")
                        nc.vector.tensor_copy(cell["p2_sb"][:, c0:c1],
                                              bank[:, c0:c1])
                    return fn

                base = cur_b[0]
                defer(base, cp(0, bsz5))
                defer(base + 1, cp(bsz5, 2 * bsz5))
                defer(base + 2, lambda: w1_mm(bank, cell["p2_sb"], bsz5,
                                              0, bsz5))
                defer(base + 3, lambda: relu_wl_out(bank, bsz5, 0, BB[1]))

            # ---- software-pipelined main loop ----
            h1_sbs = {}
            cur_b = [0]
            zt_sbs = {}
            zt_ps = emit_mma(0)
            zt_sbs[0] = ztp.tile([128, TB, 128], _bf16, name="zt_sb")
            nc.vector.tensor_copy(zt_sbs[0][:], zt_ps[:])

            for b in range(NB):
                cur_b[0] = b
                if b < 5:
                    # ramp fillers: keep PE activity dense through the
                    # DMA head so the HAM gate opens early and stays
                    # open (writes an unused corner of p2bank[1])
                    for _ in range(2):
                        nc.tensor.matmul(p2bank[1][:, 300:512],
                                         scratch[:, 0:128],
                                         scratch[:, 0:212],
                                         start=True, stop=True)
                if b + 1 < NB:
                    zt_ps = emit_mma(b + 1)
                    zt_sbs[(b + 1) % 3] = ztp.tile([128, TB, 128], _bf16,
                                                   name="zt_sb")
                    nc.vector.tensor_copy(zt_sbs[(b + 1) % 3][:], zt_ps[:])

                zt_sb = zt_sbs[b % 3]
                h1_ps = psp.tile([128, TB, F1], _f32, tag="h1", bufs=2)
                for j in range(TB):
                    nc.tensor.matmul(h1_ps[:, j, :], zt_sb[:, j, :], w0[:],
                                     start=True, stop=True)
                h1_sb = h1p.tile([128, TB, F1], _bf16)
                h1_sbs[b % 3] = h1_sb
                nc.scalar.activation(h1_sb[:, :, VSPLIT:F1],
                                     h1_ps[:, :, VSPLIT:F1], relu)
                nc.vector.tensor_scalar_max(h1_sb[:, :, 0:VSPLIT],
                                            h1_ps[:, :, 0:VSPLIT], 0.0)
                if b > 0:
                    emit_p2(b - 1)
                if b == NB - 1:
                    # first piece of block 1's p2 copy (tiles [32:48] are
                    # done after emit_p2(11); cols [80:100] are written
                    # later by p2(12) and re-copied in the tail)
                    bsz5b = (NT - BB[1]) * CP
                    c1 = (B1S - BB[1]) * CP
                    p2b_sb = p2p.tile([128, 2 * bsz5b], _bf16,
                                      name="p2b_sb")
                    nc.vector.tensor_copy(p2b_sb[:, 0:bsz5b + c1],
                                          p2bank[1][:, 0:bsz5b + c1])
                for fn in pending.pop(b, []):
                    fn()

            # ---- tail: W1 piece 1 (tiles 32:48) overlaps the last p2 ----
            cur_b[0] = NB
            bsz5b = (NT - BB[1]) * CP
            c1 = (B1S - BB[1]) * CP
            w1_mm(p2bank[1], p2b_sb, bsz5b, 0, c1)
            emit_p2(NB - 1)                      # tiles 48-51
            nc.vector.tensor_copy(p2b_sb[:, c1:bsz5b],
                                  p2bank[1][:, c1:bsz5b])
            nc.vector.tensor_copy(p2b_sb[:, bsz5b + c1:2 * bsz5b],
                                  p2bank[1][:, bsz5b + c1:2 * bsz5b])
            w1_mm(p2bank[1], p2b_sb, bsz5b, c1, bsz5b)
            relu_wl_out(p2bank[1], bsz5b, BB[1], NT)
            nc.sync.dma_start(out_d[:], out_sb[:])
            for b in sorted(list(pending)):
                for fn in pending.pop(b):
                    fn()

    nc.compile()
    return nc


def _get_nc(mode=None):
    if "v5" not in _compiled:
        _compiled["v5"] = _build_nc()
    return _compiled["v5"]


def _host_prep(x, edge_weight, W0, W1, Wlin, edge_index):
    bf = ml_dtypes.bfloat16
    src = edge_index[0].astype(np.int64)
    tgt = edge_index[1].astype(np.int64)
    b = src // K
    sl = src - b * K
    tl = tgt - (tgt // K) * K

    # dense raw adjacency per graph, indexed [b, t, s]
    idx = (b * K + tl) * K + sl
    Araw = np.bincount(idx, weights=edge_weight.astype(np.float64),
                       minlength=B * K * K).astype(np.float32).reshape(B, K, K)
    deg = Araw.sum(axis=2)                      # weighted in-degree [B, K]
    with np.errstate(divide="ignore"):
        dinv = np.where(deg > 0, 1.0 / np.sqrt(deg), 0.0).astype(np.float32)
    An = Araw * dinv[:, :, None] * dinv[:, None, :]   # [b, t, s]
    ATn = np.ascontiguousarray(An.transpose(0, 2, 1))  # [b, s, t]

    # scatter graphs into per-core padded tile slots
    SLOTS = NT * G
    ATs = np.zeros((NCORES, SLOTS, K, K), np.float32)
    ATs[:, :GPC] = ATn.reshape(NCORES, GPC, K, K)
    ATs = ATs.reshape(NCORES, NT, G, K, K)

    at = np.zeros((NCORES, NT, PP, AW), np.float32)
    bd = at[:, :, :P, :P].reshape(NCORES, NT, G, K, G, K)
    atc = np.zeros((NCORES, NT, PP, CP), np.float32)
    cent = atc[:, :, :P, :G].reshape(NCORES, NT, G, K, G)
    for g in range(G):
        bd[:, :, g, :, g, :] = ATs[:, :, g]          # block-diagonal AT
        cent[:, :, g, :, g] = ATs[:, :, g, :, 0]     # center (t_local=0) col
    # device layout [PP, NT, .]
    at = np.ascontiguousarray(at.transpose(0, 2, 1, 3).astype(bf))
    atc = np.ascontiguousarray(atc.transpose(0, 2, 1, 3).astype(bf))

    # node-major x, tiled and padded: x_nm[p, i, f] = x[i*P + p, f], p < 125
    xp = np.zeros((NCORES, NT, PP, F0), np.float32)
    xtmp = np.zeros((NCORES, NT * P, F0), np.float32)
    xtmp[:, :GPC * K] = x.reshape(NCORES, GPC * K, F0)
    xp[:, :, :P, :] = xtmp.reshape(NCORES, NT, P, F0)
    x_nm = np.ascontiguousarray(xp.transpose(0, 2, 1, 3).astype(bf))

    # packed [w1 fic0 | w1 fic1 | wl]: wpk[p, fic*256+fo] = W1[fic*128+p, fo]
    wpk = np.empty((128, 2 * F1 + 2), np.float32)
    w1p = W1.reshape(2, 128, F1).transpose(1, 0, 2)   # [128, fic, fo]
    wpk[:, 0:F1] = w1p[:, 0, :]
    wpk[:, F1:2 * F1] = w1p[:, 1, :]
    wpk[:, 2 * F1:] = Wlin.reshape(2, 128).T          # [128, foc]
    wpk = np.ascontiguousarray(wpk.astype(bf))

    in_maps = []
    for c in range(NCORES):
        in_maps.append({
            "x": x_nm[c],
            "at": np.ascontiguousarray(at[c]),
            "atc": np.ascontiguousarray(atc[c]),
            "w0": np.ascontiguousarray(W0.astype(bf)),
            "wpk": wpk,
        })
    return in_maps


def _run(inputs, mode=None, trace=False):
    nc = _get_nc()
    in_maps = _host_prep(**inputs)
    res = run_bass_kernel_spmd(nc, in_maps, core_ids=list(range(NCORES)),
                               trace=trace)
    out = np.empty((B, 1), np.float32)
    for c in range(NCORES):
        vals = res.results[c]["out"].reshape(-1)
        out[c * GPC:(c + 1) * GPC, 0] = vals[:GPC]
    return out, res


def kernel(**inputs):
    out, _ = _run(inputs, trace=False)
    return out


# revision 16
# speedup vs baseline: 1.0679x; 1.0485x over previous
"""Trainium2 Bass kernel for a 2-layer GCN over 2048 independent 25-node
KNN subgraphs (gnn_message_passing).

Strategy (v5, aggregate-first, LDWEIGHTS-port-lean, stall-free pipeline):
  - Each 25-node subgraph is independent -> the sparse aggregation is a
    dense per-graph 25x25 matmul. Host packs the normalized adjacency
    into block-diagonal 128x128 tiles (5 graphs per tile), bf16.
  - Layer 1 aggregate-first:  zT = x.T @ at;  h1 = relu(zT.T @ W0).
  - Layer-2 center aggregation: p2 = h1.T @ atc (2 matmuls/tile, 5 mov
    cols) into block-persistent PSUM banks; W1+Wlin once per block
    (2 blocks), reusing the block's own bank for h3/o after its copy.
  - Pipeline (per iteration b): mmA(b+1) -> cast zt(b+1) [vector] ->
    h1(b) -> relu(b) [scalar 224 cols + vector 32] -> p2(b-1).
    p2 is deferred one batch so it never waits on relu; the cast runs
    one batch ahead so h1 never waits on vector.  Steady state is
    LDWEIGHTS-port-bound (~16 x 95ns per 4-tile batch).
  - PSUM: zt 2 banks (bufs=2), h1 2x2 banks (bufs=2), p2 2 banks. = 8.
  - DMA: few big issues (a dma_start costs ~600-750ns on its engine);
    x + late-at on the sync HW ring, w0/atc/mid-at on the scalar HW
    ring (both stream in parallel), packed w1|wl on gpsimd.
  - 4 warm-up matmuls bridge PE activity from the init barrier until
    real data arrives, so the HAM k=4->k=8 clock gate opens during the
    DMA head.
  - Data parallel over 8 cores: 256 graphs (52 tiles) per core.
"""

import os
import sys

import ml_dtypes
import numpy as np

for _p in ("/opt/trn_rl_repo", "/opt/trn_rl_repo/concourse"):
    if _p not in sys.path:
        sys.path.insert(0, _p)

import concourse.bass as bass
import concourse.tile as tile
from concourse import bacc, mybir
from concourse.bass_utils import run_bass_kernel_spmd

NCORES = 8
B = 2048            # graphs
K = 25              # nodes per graph
N = B * K           # 51200
GPC = B // NCORES   # 256 graphs per core
G = 5               # graphs packed per PE tile
P = G * K           # 125 real partitions per tile
PP = 128            # padded partition count
NT = (GPC + G - 1) // G   # 52 tiles per core (last tile: 1 real graph)
CP = 5              # centers per tile
AW = 128            # adjacency tile width (125 block cols + 3 zero pad)
F0 = 128            # input features
F1 = 256            # hidden features
TB = 4              # tiles per batch
NB = NT // TB       # 13 batches
BB = [0, 32, 52]    # W1/Wlin block bounds (block 0 flushes mid-loop)
B1S = 48            # block-1 split: tiles [32:48] flush before p2(12)
VSPLIT = 32         # relu cols done on vector engine (rest on scalar)
NWARM = 16          # pre-loop PE warm-up matmuls (n=512).  The HAM clock
                    # gate (k=4 -> k=8) opens only after ~4.6us of dense
                    # PE activity; a sparse ramp never opens it and the
                    # whole loop runs at half clock.  The ~7us warm-up
                    # block also lets the DMA stream get ahead of the
                    # loop's tile consumption.

_f32 = mybir.dt.float32
_bf16 = mybir.dt.bfloat16

_compiled = {}


def _build_nc():
    nc = bacc.Bacc("TRN2", target_bir_lowering=False, debug=False,
                   num_devices=NCORES)

    x_d = nc.dram_tensor("x", [PP, NT, F0], _bf16, kind="ExternalInput")
    at_d = nc.dram_tensor("at", [PP, NT, AW], _bf16, kind="ExternalInput")
    atc_d = nc.dram_tensor("atc", [PP, NT, CP], _bf16, kind="ExternalInput")
    w0_d = nc.dram_tensor("w0", [F0, F1], _bf16, kind="ExternalInput")
    wpk_d = nc.dram_tensor("wpk", [128, 2 * F1 + 2], _bf16,
                           kind="ExternalInput")
    out_d = nc.dram_tensor("out", [1, NT * CP], _f32, kind="ExternalOutput")

    relu = mybir.ActivationFunctionType.Relu

    with tile.TileContext(nc) as tc:
        with (
            tc.tile_pool(name="const", bufs=1) as cpool,
            tc.tile_pool(name="ztp", bufs=3) as ztp,
            tc.tile_pool(name="h1p", bufs=3) as h1p,
            tc.tile_pool(name="p2p", bufs=2) as p2p,
            tc.tile_pool(name="h3p", bufs=2) as h3p,
            tc.tile_pool(name="outp", bufs=1) as outp,
            tc.tile_pool(name="psum", bufs=1, space=bass.MemorySpace.PSUM) as psp,
        ):
            w0 = cpool.tile([F0, F1], _bf16)
            x_sb = cpool.tile([PP, NT, F0], _bf16)
            at_sb = cpool.tile([PP, NT, AW], _bf16)
            atc_sb = cpool.tile([PP, NT, CP], _bf16)
            wpk = cpool.tile([128, 2 * F1 + 2], _bf16)
            scratch = cpool.tile([128, 512], _bf16)
            out_sb = outp.tile([1, NT * CP], _f32)

            nc.gpsimd.memset(scratch[:], 0.0)

            # ---- DMA issues (~600-750ns each on the issuing engine).
            # sync HW ring: x + tail at; scalar HW ring: w0, atc, mid at
            # (streams in parallel with sync's ring, and is done before
            # scalar's first relu); gpsimd software ring: packed w1|wl.
            nc.sync.dma_start(x_sb[:, 0:4, :], x_d[:, 0:4, :])
            nc.scalar.dma_start(w0[:], w0_d[:])
            nc.sync.dma_start(at_sb[:, 0:4, :], at_d[:, 0:4, :])
            nc.scalar.dma_start(atc_sb[:], atc_d[:])
            nc.sync.dma_start(x_sb[:, 4:18, :], x_d[:, 4:18, :])
            nc.scalar.dma_start(at_sb[:, 4:18, :], at_d[:, 4:18, :])
            nc.sync.dma_start(x_sb[:, 18:34, :], x_d[:, 18:34, :])
            nc.scalar.dma_start(at_sb[:, 18:34, :], at_d[:, 18:34, :])
            nc.sync.dma_start(x_sb[:, 34:52, :], x_d[:, 34:52, :])
            nc.sync.dma_start(at_sb[:, 34:52, :], at_d[:, 34:52, :])
            nc.gpsimd.dma_start(wpk[:], wpk_d[:])

            # ---- PSUM layout (8 banks):
            #   zt  tag: 2 x [128,4,128] f32 (1 bank each)
            #   h1  tag: 2 x [128,4,256] f32 (2 banks each; warm shares)
            #   p2  tag: 2 x [128,512] f32 (block p2 + h3/o reuse)
            p2bank = [psp.tile([128, 512], _f32, tag="p2", bufs=2,
                               name=f"p2bank{k}") for k in range(2)]

            warm_ps = psp.tile([128, TB, F1], _f32, tag="h1", bufs=2)

            def warm(n):
                # dense PE activity to open (and hold open) the HAM
                # clock gate; also fills would-be DMA stalls in the ramp
                for _ in range(n):
                    nc.tensor.matmul(warm_ps[:, 0:2, :], scratch[:, 0:128],
                                     scratch[:], start=True, stop=True)

            warm(16)

            pending = {}

            def defer(b, fn):
                pending.setdefault(b, []).append(fn)

            def emit_mma(b):
                zt_ps = psp.tile([128, TB, 128], _f32, tag="zt", bufs=2)
                for j in range(TB):
                    i = b * TB + j
                    nc.tensor.matmul(zt_ps[:, j, :], x_sb[:, i, :],
                                     at_sb[:, i, :], start=True, stop=True)
                return zt_ps

            def emit_p2(b):
                # layer-2 center aggregation for batch b (deferred one
                # batch so it never waits on relu)
                for jj in range(TB):
                    i = b * TB + jj
                    k = 0 if i < BB[1] else 1
                    bsz5 = (BB[k + 1] - BB[k]) * CP
                    off = (i - BB[k]) * CP
                    for fic in range(2):
                        nc.tensor.matmul(
                            p2bank[k][:, fic * bsz5 + off:
                                      fic * bsz5 + off + CP],
                            h1_sbs[b % 3][:, jj, fic * 128:(fic + 1) * 128],
                            atc_sb[:, i, :],
                            start=True, stop=True)
                    if i + 1 == BB[1]:
                        emit_flush0()

            def w1_mm(bank, p2_sb, bsz5, clo, chi):
                # h3[:, foc, clo:chi] += W1 chunks @ p2 (h3 reuses the
                # block's own bank; its p2 cols are dead after the copy)
                for foc in range(2):
                    for fic in range(2):
                        nc.tensor.matmul(
                            bank[:, foc * bsz5 + clo:foc * bsz5 + chi],
                            wpk[:, fic * F1 + foc * 128:
                                fic * F1 + (foc + 1) * 128],
                            p2_sb[:, fic * bsz5 + clo:fic * bsz5 + chi],
                            start=(fic == 0), stop=(fic == 1))

            def relu_wl_out(bank, bsz5, lo, hi):
                h3_sb = h3p.tile([128, 2 * bsz5], _bf16, name="h3_sb")
                nc.scalar.activation(h3_sb[:], bank[:, 0:2 * bsz5], relu)
                o_ps = bank[0:1, 2 * bsz5:3 * bsz5]
                for foc in range(2):
                    nc.tensor.matmul(o_ps, wpk[:, 2 * F1 + foc:
                                               2 * F1 + foc + 1],
                                     h3_sb[:, foc * bsz5:(foc + 1) * bsz5],
                                     start=(foc == 0), stop=(foc == 1))
                nc.vector.tensor_copy(out_sb[0:1, lo * CP:hi * CP], o_ps)

            def emit_flush0():
                # block 0 complete: pipeline split-copy -> W1 -> out over
                # the next iterations, all inside the loop
                bsz5 = BB[1] * CP
                bank = p2bank[0]
                cell = {}

                def cp(c0, c1):
                    def fn():
                        if "p2_sb" not in cell:
                            cell["p2_sb"] = p2p.tile([128, 2 * bsz5], _bf16,
                                                     name="p2a_sb")
                        nc.vector.tensor_copy(cell["p2_sb"][:, c0:c1],
                                              bank[:, c0:c1])
                    return fn

                base = cur_b[0]
                defer(base, cp(0, bsz5))
                defer(base + 1, cp(bsz5, 2 * bsz5))
                defer(base + 2, lambda: w1_mm(bank, cell["p2_sb"], bsz5,
                                              0, bsz5))
                defer(base + 3, lambda: relu_wl_out(bank, bsz5, 0, BB[1]))

            # ---- software-pipelined main loop ----
            h1_sbs = {}
            cur_b = [0]
            zt_sbs = {}
            zt_ps = emit_mma(0)
            zt_sbs[0] = ztp.tile([128, TB, 128], _bf16, name="zt_sb")
            nc.vector.tensor_copy(zt_sbs[0][:], zt_ps[:])

            for b in range(NB):
                cur_b[0] = b
                if b + 1 < NB:
                    zt_ps = emit_mma(b + 1)
                    zt_sbs[(b + 1) % 3] = ztp.tile([128, TB, 128], _bf16,
                                                   name="zt_sb")
                    nc.vector.tensor_copy(zt_sbs[(b + 1) % 3][:], zt_ps[:])

                zt_sb = zt_sbs[b % 3]
                h1_ps = psp.tile([128, TB, F1], _f32, tag="h1", bufs=2)
                for j in range(TB):
                    nc.tensor.matmul(h1_ps[:, j, :], zt_sb[:, j, :], w0[:],
                                     start=True, stop=True)
                h1_sb = h1p.tile([128, TB, F1], _bf16)
                h1_sbs[b % 3] = h1_sb
                nc.scalar.activation(h1_sb[:, :, VSPLIT:F1],
                                     h1_ps[:, :, VSPLIT:F1], relu)
                nc.vector.tensor_scalar_max(h1_sb[:, :, 0:VSPLIT],
                                            h1_ps[:, :, 0:VSPLIT], 0.0)
                if b > 0:
                    emit_p2(b - 1)
                if b == NB - 1:
                    # first piece of block 1's p2 copy (tiles [32:48] are
                    # done after emit_p2(11); cols [80:100] are written
                    # later by p2(12) and re-copied in the tail)
                    bsz5b = (NT - BB[1]) * CP
                    c1 = (B1S - BB[1]) * CP
                    p2b_sb = p2p.tile([128, 2 * bsz5b], _bf16,
                                      name="p2b_sb")
                    nc.vector.tensor_copy(p2b_sb[:, 0:bsz5b + c1],
                                          p2bank[1][:, 0:bsz5b + c1])
                for fn in pending.pop(b, []):
                    fn()

            # ---- tail: W1 piece 1 (tiles 32:48) overlaps the last p2 ----
            cur_b[0] = NB
            bsz5b = (NT - BB[1]) * CP
            c1 = (B1S - BB[1]) * CP
            w1_mm(p2bank[1], p2b_sb, bsz5b, 0, c1)
            emit_p2(NB - 1)                      # tiles 48-51
            nc.vector.tensor_copy(p2b_sb[:, c1:bsz5b],
                                  p2bank[1][:, c1:bsz5b])
            nc.vector.tensor_copy(p2b_sb[:, bsz5b + c1:2 * bsz5b],
                                  p2bank[1][:, bsz5b + c1:2 * bsz5b])
            w1_mm(p2bank[1], p2b_sb, bsz5b, c1, bsz5b)
            relu_wl_out(p2bank[1], bsz5b, BB[1], NT)
            nc.sync.dma_start(out_d[:], out_sb[:])
            for b in sorted(list(pending)):
                for fn in pending.pop(b):
                    fn()

    nc.compile()
    return nc


def _get_nc(mode=None):
    if "v5" not in _compiled:
        _compiled["v5"] = _build_nc()
    return _compiled["v5"]


def _host_prep(x, edge_weight, W0, W1, Wlin, edge_index):
    bf = ml_dtypes.bfloat16
    src = edge_index[0].astype(np.int64)
    tgt = edge_index[1].astype(np.int64)
    b = src // K
    sl = src - b * K
    tl = tgt - (tgt // K) * K

    # dense raw adjacency per graph, indexed [b, t, s]
    idx = (b * K + tl) * K + sl
    Araw = np.bincount(idx, weights=edge_weight.astype(np.float64),
                       minlength=B * K * K).astype(np.float32).reshape(B, K, K)
    deg = Araw.sum(axis=2)                      # weighted in-degree [B, K]
    with np.errstate(divide="ignore"):
        dinv = np.where(deg > 0, 1.0 / np.sqrt(deg), 0.0).astype(np.float32)
    An = Araw * dinv[:, :, None] * dinv[:, None, :]   # [b, t, s]
    ATn = np.ascontiguousarray(An.transpose(0, 2, 1))  # [b, s, t]

    # scatter graphs into per-core padded tile slots
    SLOTS = NT * G
    ATs = np.zeros((NCORES, SLOTS, K, K), np.float32)
    ATs[:, :GPC] = ATn.reshape(NCORES, GPC, K, K)
    ATs = ATs.reshape(NCORES, NT, G, K, K)

    at = np.zeros((NCORES, NT, PP, AW), np.float32)
    bd = at[:, :, :P, :P].reshape(NCORES, NT, G, K, G, K)
    atc = np.zeros((NCORES, NT, PP, CP), np.float32)
    cent = atc[:, :, :P, :G].reshape(NCORES, NT, G, K, G)
    for g in range(G):
        bd[:, :, g, :, g, :] = ATs[:, :, g]          # block-diagonal AT
        cent[:, :, g, :, g] = ATs[:, :, g, :, 0]     # center (t_local=0) col
    # device layout [PP, NT, .]
    at = np.ascontiguousarray(at.transpose(0, 2, 1, 3).astype(bf))
    atc = np.ascontiguousarray(atc.transpose(0, 2, 1, 3).astype(bf))

    # node-major x, tiled and padded: x_nm[p, i, f] = x[i*P + p, f], p < 125
    xp = np.zeros((NCORES, NT, PP, F0), np.float32)
    xtmp = np.zeros((NCORES, NT * P, F0), np.float32)
    xtmp[:, :GPC * K] = x.reshape(NCORES, GPC * K, F0)
    xp[:, :, :P, :] = xtmp.reshape(NCORES, NT, P, F0)
    x_nm = np.ascontiguousarray(xp.transpose(0, 2, 1, 3).astype(bf))

    # packed [w1 fic0 | w1 fic1 | wl]: wpk[p, fic*256+fo] = W1[fic*128+p, fo]
    wpk = np.empty((128, 2 * F1 + 2), np.float32)
    w1p = W1.reshape(2, 128, F1).transpose(1, 0, 2)   # [128, fic, fo]
    wpk[:, 0:F1] = w1p[:, 0, :]
    wpk[:, F1:2 * F1] = w1p[:, 1, :]
    wpk[:, 2 * F1:] = Wlin.reshape(2, 128).T          # [128, foc]
    wpk = np.ascontiguousarray(wpk.astype(bf))

    in_maps = []
    for c in range(NCORES):
        in_maps.append({
            "x": x_nm[c],
            "at": np.ascontiguousarray(at[c]),
            "atc": np.ascontiguousarray(atc[c]),
            "w0": np.ascontiguousarray(W0.astype(bf)),
            "wpk": wpk,
        })
    return in_maps


def _run(inputs, mode=None, trace=False):
    nc = _get_nc()
    in_maps = _host_prep(**inputs)
    res = run_bass_kernel_spmd(nc, in_maps, core_ids=list(range(NCORES)),
                               trace=trace)
    out = np.empty((B, 1), np.float32)
    for c in range(NCORES):
        vals = res.results[c]["out"].reshape(-1)
        out[c * GPC:(c + 1) * GPC, 0] = vals[:GPC]
    return out, res


def kernel(**inputs):
    out, _ = _run(inputs, trace=False)
    return out


# revision 23
# speedup vs baseline: 1.1341x; 1.0621x over previous
"""Trainium2 Bass kernel for a 2-layer GCN over 2048 independent 25-node
KNN subgraphs (gnn_message_passing).

Strategy (v5, aggregate-first, LDWEIGHTS-port-lean, stall-free pipeline):
  - Each 25-node subgraph is independent -> the sparse aggregation is a
    dense per-graph 25x25 matmul. Host packs the normalized adjacency
    into block-diagonal 128x128 tiles (5 graphs per tile), bf16.
  - Layer 1 aggregate-first:  zT = x.T @ at;  h1 = relu(zT.T @ W0).
  - Layer-2 center aggregation: p2 = h1.T @ atc (2 matmuls/tile, 5 mov
    cols) into block-persistent PSUM banks; W1+Wlin once per block
    (2 blocks), reusing the block's own bank for h3/o after its copy.
  - Pipeline (per iteration b): mmA(b+1) -> cast zt(b+1) [vector] ->
    h1(b) -> relu(b) [scalar 224 cols + vector 32] -> p2(b-1).
    p2 is deferred one batch so it never waits on relu; the cast runs
    one batch ahead so h1 never waits on vector.  Steady state is
    LDWEIGHTS-port-bound (~16 x 95ns per 4-tile batch).
  - PSUM: zt 2 banks (bufs=2), h1 2x2 banks (bufs=2), p2 2 banks. = 8.
  - DMA: few big issues (a dma_start costs ~600-750ns on its engine);
    x + late-at on the sync HW ring, w0/atc/mid-at on the scalar HW
    ring (both stream in parallel), packed w1|wl on gpsimd.
  - 4 warm-up matmuls bridge PE activity from the init barrier until
    real data arrives, so the HAM k=4->k=8 clock gate opens during the
    DMA head.
  - Data parallel over 8 cores: 256 graphs (52 tiles) per core.
"""

import os
import sys

import ml_dtypes
import numpy as np

for _p in ("/opt/trn_rl_repo", "/opt/trn_rl_repo/concourse"):
    if _p not in sys.path:
        sys.path.insert(0, _p)

import concourse.bass as bass
import concourse.tile as tile
from concourse import bacc, mybir
from concourse.bass_utils import run_bass_kernel_spmd

NCORES = 8
B = 2048            # graphs
K = 25              # nodes per graph
N = B * K           # 51200
GPC = B // NCORES   # 256 graphs per core
G = 5               # graphs packed per PE tile
P = G * K           # 125 real partitions per tile
PP = 128            # padded partition count
NT = (GPC + G - 1) // G   # 52 tiles per core (last tile: 1 real graph)
CP = 5              # centers per tile
AW = 128            # adjacency tile width (125 block cols + 3 zero pad)
F0 = 128            # input features
F1 = 256            # hidden features
TB = 4              # tiles per batch
NB = NT // TB       # 13 batches
BB = [0, 28, 52]    # W1/Wlin block bounds (block 0 flushes mid-loop)
B1S = 48            # block-1 split: tiles [28:48] flush before p2(12)
VSPLIT = 32         # relu cols done on vector engine (rest on scalar)
NWARM = 16          # pre-loop PE warm-up matmuls (n=512).  The HAM clock
                    # gate (k=4 -> k=8) opens only after ~4.6us of dense
                    # PE activity; a sparse ramp never opens it and the
                    # whole loop runs at half clock.  The ~7us warm-up
                    # block also lets the DMA stream get ahead of the
                    # loop's tile consumption.

_f32 = mybir.dt.float32
_bf16 = mybir.dt.bfloat16

_compiled = {}


def _build_nc():
    nc = bacc.Bacc("TRN2", target_bir_lowering=False, debug=False,
                   num_devices=NCORES)

    x_d = nc.dram_tensor("x", [PP, NT, F0], _bf16, kind="ExternalInput")
    at_d = nc.dram_tensor("at", [PP, NT, AW], _bf16, kind="ExternalInput")
    atc_d = nc.dram_tensor("atc", [PP, NT, CP], _bf16, kind="ExternalInput")
    w0_d = nc.dram_tensor("w0", [F0, F1], _bf16, kind="ExternalInput")
    wpk_d = nc.dram_tensor("wpk", [128, 2 * F1 + 2], _bf16,
                           kind="ExternalInput")
    out_d = nc.dram_tensor("out", [1, NT * CP], _f32, kind="ExternalOutput")

    relu = mybir.ActivationFunctionType.Relu

    with tile.TileContext(nc) as tc:
        with (
            tc.tile_pool(name="const", bufs=1) as cpool,
            tc.tile_pool(name="ztp", bufs=3) as ztp,
            tc.tile_pool(name="h1p", bufs=3) as h1p,
            tc.tile_pool(name="p2p", bufs=2) as p2p,
            tc.tile_pool(name="h3p", bufs=2) as h3p,
            tc.tile_pool(name="outp", bufs=1) as outp,
            tc.tile_pool(name="psum", bufs=1, space=bass.MemorySpace.PSUM) as psp,
        ):
            w0 = cpool.tile([F0, F1], _bf16)
            x_sb = cpool.tile([PP, NT, F0], _bf16)
            at_sb = cpool.tile([PP, NT, AW], _bf16)
            atc_sb = cpool.tile([PP, NT, CP], _bf16)
            wpk = cpool.tile([128, 2 * F1 + 2], _bf16)
            scratch = cpool.tile([128, 512], _bf16)
            out_sb = outp.tile([1, NT * CP], _f32)

            nc.gpsimd.memset(scratch[:], 0.0)

            # ---- DMA issues (~600-750ns each on the issuing engine).
            # sync HW ring: x + tail at; scalar HW ring: w0, atc, mid at
            # (streams in parallel with sync's ring, and is done before
            # scalar's first relu); gpsimd software ring: packed w1|wl.
            nc.sync.dma_start(x_sb[:, 0:4, :], x_d[:, 0:4, :])
            nc.scalar.dma_start(w0[:], w0_d[:])
            nc.sync.dma_start(at_sb[:, 0:4, :], at_d[:, 0:4, :])
            nc.scalar.dma_start(atc_sb[:], atc_d[:])
            nc.sync.dma_start(x_sb[:, 4:18, :], x_d[:, 4:18, :])
            nc.scalar.dma_start(at_sb[:, 4:18, :], at_d[:, 4:18, :])
            nc.sync.dma_start(x_sb[:, 18:34, :], x_d[:, 18:34, :])
            nc.scalar.dma_start(at_sb[:, 18:34, :], at_d[:, 18:34, :])
            nc.sync.dma_start(x_sb[:, 34:52, :], x_d[:, 34:52, :])
            nc.sync.dma_start(at_sb[:, 34:52, :], at_d[:, 34:52, :])
            nc.gpsimd.dma_start(wpk[:], wpk_d[:])

            # ---- PSUM layout (8 banks):
            #   zt  tag: 2 x [128,4,128] f32 (1 bank each)
            #   h1  tag: 2 x [128,4,256] f32 (2 banks each; warm shares)
            #   p2  tag: 2 x [128,512] f32 (block p2 + h3/o reuse)
            p2bank = [psp.tile([128, 512], _f32, tag="p2", bufs=2,
                               name=f"p2bank{k}") for k in range(2)]

            warm_ps = psp.tile([128, TB, F1], _f32, tag="h1", bufs=2)

            def warm(n):
                # dense PE activity to open (and hold open) the HAM
                # clock gate; also fills would-be DMA stalls in the ramp
                for _ in range(n):
                    nc.tensor.matmul(warm_ps[:, 0:2, :], scratch[:, 0:128],
                                     scratch[:], start=True, stop=True)

            warm(16)

            pending = {}

            def defer(b, fn):
                pending.setdefault(b, []).append(fn)

            def emit_mma(b):
                zt_ps = psp.tile([128, TB, 128], _f32, tag="zt", bufs=2)
                for j in range(TB):
                    i = b * TB + j
                    nc.tensor.matmul(zt_ps[:, j, :], x_sb[:, i, :],
                                     at_sb[:, i, :], start=True, stop=True)
                return zt_ps

            def emit_p2(b):
                # layer-2 center aggregation for batch b (deferred one
                # batch so it never waits on relu)
                for jj in range(TB):
                    i = b * TB + jj
                    k = 0 if i < BB[1] else 1
                    bsz5 = (BB[k + 1] - BB[k]) * CP
                    off = (i - BB[k]) * CP
                    for fic in range(2):
                        nc.tensor.matmul(
                            p2bank[k][:, fic * bsz5 + off:
                                      fic * bsz5 + off + CP],
                            h1_sbs[b % 3][:, jj, fic * 128:(fic + 1) * 128],
                            atc_sb[:, i, :],
                            start=True, stop=True)
                    if i + 1 == BB[1]:
                        emit_flush0()

            def w1_mm(bank, p2_sb, bsz5, clo, chi):
                # h3[:, foc, clo:chi] += W1 chunks @ p2 (h3 reuses the
                # block's own bank; its p2 cols are dead after the copy)
                for foc in range(2):
                    for fic in range(2):
                        nc.tensor.matmul(
                            bank[:, foc * bsz5 + clo:foc * bsz5 + chi],
                            wpk[:, fic * F1 + foc * 128:
                                fic * F1 + (foc + 1) * 128],
                            p2_sb[:, fic * bsz5 + clo:fic * bsz5 + chi],
                            start=(fic == 0), stop=(fic == 1))

            def wl_out(bank, h3_sb, bsz5, lo, hi):
                o_ps = bank[0:1, 2 * bsz5:3 * bsz5]
                for foc in range(2):
                    nc.tensor.matmul(o_ps, wpk[:, 2 * F1 + foc:
                                               2 * F1 + foc + 1],
                                     h3_sb[:, foc * bsz5:(foc + 1) * bsz5],
                                     start=(foc == 0), stop=(foc == 1))
                nc.vector.tensor_copy(out_sb[0:1, lo * CP:hi * CP], o_ps)

            def emit_flush0():
                # block 0 complete: pipeline split-copy -> W1 -> relu
                # halves -> Wlin|out over the next 6 iterations; every
                # piece is sized to keep vector/scalar under their
                # per-iteration budget (any PE stall risks a ~3.4us HAM
                # re-throttle to half clock)
                bsz5 = BB[1] * CP
                bank = p2bank[0]
                cell = {"p2_sb": p2p.tile([128, 2 * bsz5], _bf16,
                                          name="p2a_sb"),
                        "h3_sb": h3p.tile([128, 2 * bsz5], _bf16,
                                          name="h3a_sb")}

                def cp(c0, c1):
                    return lambda: nc.vector.tensor_copy(
                        cell["p2_sb"][:, c0:c1], bank[:, c0:c1])

                def h3half(c0, c1):
                    return lambda: nc.vector.tensor_scalar_max(
                        cell["h3_sb"][:, c0:c1], bank[:, c0:c1], 0.0)

                base = cur_b[0]
                defer(base, cp(0, bsz5))
                defer(base + 1, cp(bsz5, 2 * bsz5))
                defer(base + 2, lambda: w1_mm(bank, cell["p2_sb"], bsz5,
                                              0, bsz5))
                defer(base + 3, h3half(0, bsz5))
                defer(base + 4, h3half(bsz5, 2 * bsz5))
                defer(base + 5, lambda: wl_out(bank, cell["h3_sb"], bsz5,
                                               0, BB[1]))

            # ---- software-pipelined main loop ----
            h1_sbs = {}
            cur_b = [0]
            zt_sbs = {}
            zt_ps = emit_mma(0)
            zt_sbs[0] = ztp.tile([128, TB, 128], _bf16, name="zt_sb")
            nc.vector.tensor_copy(zt_sbs[0][:], zt_ps[:])

            for b in range(NB):
                cur_b[0] = b
                if b + 1 < NB:
                    zt_ps = emit_mma(b + 1)
                    zt_sbs[(b + 1) % 3] = ztp.tile([128, TB, 128], _bf16,
                                                   name="zt_sb")
                    nc.vector.tensor_copy(zt_sbs[(b + 1) % 3][:], zt_ps[:])

                zt_sb = zt_sbs[b % 3]
                h1_ps = psp.tile([128, TB, F1], _f32, tag="h1", bufs=2)
                for j in range(TB):
                    nc.tensor.matmul(h1_ps[:, j, :], zt_sb[:, j, :], w0[:],
                                     start=True, stop=True)
                h1_sb = h1p.tile([128, TB, F1], _bf16)
                h1_sbs[b % 3] = h1_sb
                nc.scalar.activation(h1_sb[:, :, VSPLIT:F1],
                                     h1_ps[:, :, VSPLIT:F1], relu)
                nc.vector.tensor_scalar_max(h1_sb[:, :, 0:VSPLIT],
                                            h1_ps[:, :, 0:VSPLIT], 0.0)
                if b > 0:
                    emit_p2(b - 1)
                if b == NB - 1:
                    # first piece of block 1's p2 copy (tiles [32:48] are
                    # done after emit_p2(11); cols [80:100] are written
                    # later by p2(12) and re-copied in the tail)
                    bsz5b = (NT - BB[1]) * CP
                    c1 = (B1S - BB[1]) * CP
                    p2b_sb = p2p.tile([128, 2 * bsz5b], _bf16,
                                      name="p2b_sb")
                    nc.vector.tensor_copy(p2b_sb[:, 0:bsz5b + c1],
                                          p2bank[1][:, 0:bsz5b + c1])
                for fn in pending.pop(b, []):
                    fn()

            # ---- tail: W1 piece 1 (tiles 32:48) overlaps the last p2 ----
            cur_b[0] = NB
            bsz5b = (NT - BB[1]) * CP
            c1 = (B1S - BB[1]) * CP
            w1_mm(p2bank[1], p2b_sb, bsz5b, 0, c1)
            emit_p2(NB - 1)                      # tiles 48-51
            nc.vector.tensor_copy(p2b_sb[:, c1:bsz5b],
                                  p2bank[1][:, c1:bsz5b])
            nc.vector.tensor_copy(p2b_sb[:, bsz5b + c1:2 * bsz5b],
                                  p2bank[1][:, bsz5b + c1:2 * bsz5b])
            w1_mm(p2bank[1], p2b_sb, bsz5b, c1, bsz5b)
            h3b_sb = h3p.tile([128, 2 * bsz5b], _bf16, name="h3b_sb")
            nc.scalar.activation(h3b_sb[:], p2bank[1][:, 0:2 * bsz5b], relu)
            wl_out(p2bank[1], h3b_sb, bsz5b, BB[1], NT)
            nc.sync.dma_start(out_d[:], out_sb[:])
            for b in sorted(list(pending)):
                for fn in pending.pop(b):
                    fn()

    nc.compile()
    return nc


def _get_nc(mode=None):
    if "v5" not in _compiled:
        _compiled["v5"] = _build_nc()
    return _compiled["v5"]


def _host_prep(x, edge_weight, W0, W1, Wlin, edge_index):
    bf = ml_dtypes.bfloat16
    src = edge_index[0].astype(np.int64)
    tgt = edge_index[1].astype(np.int64)
    b = src // K
    sl = src - b * K
    tl = tgt - (tgt // K) * K

    # dense raw adjacency per graph, indexed [b, t, s]
    idx = (b * K + tl) * K + sl
    Araw = np.bincount(idx, weights=edge_weight.astype(np.float64),
                       minlength=B * K * K).astype(np.float32).reshape(B, K, K)
    deg = Araw.sum(axis=2)                      # weighted in-degree [B, K]
    with np.errstate(divide="ignore"):
        dinv = np.where(deg > 0, 1.0 / np.sqrt(deg), 0.0).astype(np.float32)
    An = Araw * dinv[:, :, None] * dinv[:, None, :]   # [b, t, s]
    ATn = np.ascontiguousarray(An.transpose(0, 2, 1))  # [b, s, t]

    # scatter graphs into per-core padded tile slots
    SLOTS = NT * G
    ATs = np.zeros((NCORES, SLOTS, K, K), np.float32)
    ATs[:, :GPC] = ATn.reshape(NCORES, GPC, K, K)
    ATs = ATs.reshape(NCORES, NT, G, K, K)

    at = np.zeros((NCORES, NT, PP, AW), np.float32)
    bd = at[:, :, :P, :P].reshape(NCORES, NT, G, K, G, K)
    atc = np.zeros((NCORES, NT, PP, CP), np.float32)
    cent = atc[:, :, :P, :G].reshape(NCORES, NT, G, K, G)
    for g in range(G):
        bd[:, :, g, :, g, :] = ATs[:, :, g]          # block-diagonal AT
        cent[:, :, g, :, g] = ATs[:, :, g, :, 0]     # center (t_local=0) col
    # device layout [PP, NT, .]
    at = np.ascontiguousarray(at.transpose(0, 2, 1, 3).astype(bf))
    atc = np.ascontiguousarray(atc.transpose(0, 2, 1, 3).astype(bf))

    # node-major x, tiled and padded: x_nm[p, i, f] = x[i*P + p, f], p < 125
    xp = np.zeros((NCORES, NT, PP, F0), np.float32)
    xtmp = np.zeros((NCORES, NT * P, F0), np.float32)
    xtmp[:, :GPC * K] = x.reshape(NCORES, GPC * K, F0)
    xp[:, :, :P, :] = xtmp.reshape(NCORES, NT, P, F0)
    x_nm = np.ascontiguousarray(xp.transpose(0, 2, 1, 3).astype(bf))

    # packed [w1 fic0 | w1 fic1 | wl]: wpk[p, fic*256+fo] = W1[fic*128+p, fo]
    wpk = np.empty((128, 2 * F1 + 2), np.float32)
    w1p = W1.reshape(2, 128, F1).transpose(1, 0, 2)   # [128, fic, fo]
    wpk[:, 0:F1] = w1p[:, 0, :]
    wpk[:, F1:2 * F1] = w1p[:, 1, :]
    wpk[:, 2 * F1:] = Wlin.reshape(2, 128).T          # [128, foc]
    wpk = np.ascontiguousarray(wpk.astype(bf))

    in_maps = []
    for c in range(NCORES):
        in_maps.append({
            "x": x_nm[c],
            "at": np.ascontiguousarray(at[c]),
            "atc": np.ascontiguousarray(atc[c]),
            "w0": np.ascontiguousarray(W0.astype(bf)),
            "wpk": wpk,
        })
    return in_maps


def _run(inputs, mode=None, trace=False):
    nc = _get_nc()
    in_maps = _host_prep(**inputs)
    res = run_bass_kernel_spmd(nc, in_maps, core_ids=list(range(NCORES)),
                               trace=trace)
    out = np.empty((B, 1), np.float32)
    for c in range(NCORES):
        vals = res.results[c]["out"].reshape(-1)
        out[c * GPC:(c + 1) * GPC, 0] = vals[:GPC]
    return out, res


def kernel(**inputs):
    out, _ = _run(inputs, trace=False)
    return out
